# revision 1
# baseline (speedup 1.0000x reference)
"""Self-contained Trainium2 (Bass) kernel for a 3-conv GCN encoder.

reference math (PyG GCNConv with edge weights, symmetric norm, self loops):
    deg[t]  = 1 + sum_{e: col[e]=t} ew[e]
    dinv    = deg ** -0.5
    agg(X)[t] = dinv[t] * ( dinv[t]*X[t] + sum_{e->t} ew[e]*dinv[src]*X[src] )
    h  = relu(agg(x) @ W1 + b1)            ->  hs := dinv * h
    mu = agg(h) @ Wmu + bmu ; logstd = agg(h) @ Wls + bls

Distribution: nodes target-sharded across 8 cores. Per-edge source rows are
fetched with int16 `dma_gather` from a replicated table (AllGather of the
per-shard scaled features xs = dinv*x, then hs). The int16 limit (32767) is
handled by splitting the table into NR row-ranges; per range, targets are
re-compacted and degree-sorted so slot-major grids have ~zero padding, and
per-range partial sums are re-merged by a tiny int16-clean gather pass.
"""

import numpy as np


def _make_cfg(n, e, p, nr, f=128, h=128, o=64, cols_per_call=16, mb_batch=8,
              grid_bufs=3, mg_bufs=2, work_bufs=4):
    sh = n // p
    nb = -(-sh // 128)
    npad = nb * 128
    tbl = p * npad
    assert tbl % nr == 0
    rs = tbl // nr
    assert rs <= 32600, (rs, "int16 gather range too large")
    return dict(n=n, e=e, p=p, f=f, h=h, o=o, sh=sh, nb=nb, npad=npad,
                tbl=tbl, nr=nr, rs=rs, cols_per_call=cols_per_call,
                mb_batch=mb_batch, grid_bufs=grid_bufs, mg_bufs=mg_bufs,
                work_bufs=work_bufs)


CFG_PROD = dict(n=100000, e=1600000, p=8, nr=4)


# ----------------------------------------------------------------------------
# walrus compat shim: this env's walrus rejects >1 sync-wait per instruction
# (and any wait on InstDrain); hoist excess waits onto InstEventSemaphore.
# ----------------------------------------------------------------------------

def _split_excess_waits(nc, max_inline=1):
    import concourse.mybir as mybir
    n_moved = 0
    for fn in nc.m.functions:
        for bb in fn.blocks:
            new_insts = []
            for inst in bb.instructions:
                si = inst.sync_info
                if si is not None and si.on_wait:
                    keep = 0 if isinstance(inst, mybir.InstDrain) else max_inline
                    if isinstance(inst, mybir.InstEventSemaphore):
                        keep = max(keep, 1)
                    waits = list(si.on_wait)
                    if len(waits) > keep:
                        hoist = waits[:-keep] if keep else waits
                        inline = waits[-keep:] if keep else []
                        for w in hoist:
                            ev = mybir.InstEventSemaphore(
                                name=nc.get_next_instruction_name(), ins=[], outs=[])
                            ev.engine = inst.engine
                            ev.sync_info = mybir.SyncInfo(on_wait=[w], on_update=[])
                            new_insts.append(ev)
                            n_moved += 1
                        si.on_wait = inline
                new_insts.append(inst)
            bb.instructions[:] = new_insts
    return n_moved


# ----------------------------------------------------------------------------
# host preprocessing (pure index/shuffle work; all FP math stays on device)
# ----------------------------------------------------------------------------

def _wrap16(idxs):
    """int16 index stream -> [128, n/16] tile (16-wrapped, 8x replicated)."""
    n = len(idxs)
    assert n % 16 == 0
    t = np.zeros((128, n // 16), dtype=np.int16)
    blk = idxs.reshape(n // 16, 16).T.astype(np.int16)
    for k in range(8):
        t[16 * k:16 * (k + 1), :] = blk
    return t


def _slot_ranks(sorted_keys):
    """for a sorted int array, rank of each element within its value-group."""
    n = len(sorted_keys)
    if n == 0:
        return np.zeros(0, dtype=np.int64)
    starts = np.r_[0, np.flatnonzero(np.diff(sorted_keys)) + 1]
    group_start = np.repeat(starts, np.diff(np.r_[starts, n]))
    return np.arange(n) - group_start


def _preprocess(cfg, x, edge_index, edge_attr, W1, b1, Wmu, bmu, Wls, bls):
    p = cfg["p"]
    sh, nb, npad, nr, rs = cfg["sh"], cfg["nb"], cfg["npad"], cfg["nr"], cfg["rs"]

    row = np.asarray(edge_index[0], dtype=np.int64)
    col = np.asarray(edge_index[1], dtype=np.int64)
    ew = np.asarray(edge_attr, dtype=np.float32)
    x = np.asarray(x, dtype=np.float32)

    # per-shard target permutation (by total in-degree, desc) -------------
    shard_of = col // sh
    tloc = col - shard_of * sh
    pis, poss = [], []
    for c in range(p):
        deg_cnt = np.bincount(tloc[shard_of == c], minlength=sh)
        pi = np.argsort(-deg_cnt, kind="stable")
        pi_full = np.concatenate([pi, np.arange(sh, npad)])
        pos = np.empty(npad, dtype=np.int64)
        pos[pi_full] = np.arange(npad)
        pis.append(pi_full)
        poss.append(pos)

    # table row of each edge's source (shard-major, per-shard pi order)
    src_shard = row // sh
    src_loc = row - src_shard * sh
    xrow = np.empty(len(row), dtype=np.int64)
    for c in range(p):
        m = src_shard == c
        xrow[m] = c * npad + poss[c][src_loc[m]]
    rng_of = xrow // rs
    lidx = xrow - rng_of * rs

    per_core = []
    for c in range(p):
        m = shard_of == c
        per_core.append(dict(q=poss[c][tloc[m]], r=rng_of[m],
                             li=lidx[m], ew=ew[m]))

    # per-(core, range) compact ordering; uniformized shapes --------------
    ncb = np.ones(nr, dtype=np.int64)
    Ls = [[None] * nr for _ in range(p)]
    orders = [[None] * nr for _ in range(p)]
    cpos = [[None] * nr for _ in range(p)]
    for c in range(p):
        pc = per_core[c]
        for r in range(nr):
            L = np.bincount(pc["q"][pc["r"] == r], minlength=npad)
            order = np.argsort(-L, kind="stable")
            cp = np.empty(npad, dtype=np.int64)
            cp[order] = np.arange(npad)
            Ls[c][r], orders[c][r], cpos[c][r] = L, order, cp
            nnz = int((L > 0).sum())
            ncb[r] = max(ncb[r], max(1, -(-nnz // 128)))

    S = [np.zeros(int(ncb[r]), dtype=np.int64) for r in range(nr)]
    for r in range(nr):
        for c in range(p):
            Lsort = Ls[c][r][orders[c][r]]
            for cb in range(int(ncb[r])):
                blk = Lsort[cb * 128:(cb + 1) * 128]
                if len(blk):
                    S[r][cb] = max(S[r][cb], int(blk.max()))
    gofs_r = np.concatenate([[0], np.cumsum([int(S[r].sum()) for r in range(nr)])])
    gcols = int(gofs_r[-1])

    TS = np.zeros(nb, dtype=np.int64)
    for c in range(p):
        Lt = np.bincount(per_core[c]["q"], minlength=npad)
        for b in range(nb):
            TS[b] = max(TS[b], int(Lt[b * 128:(b + 1) * 128].max()))
    tcols = max(1, int(TS.sum()))

    # per-core device arrays ---------------------------------------------
    in_maps = []
    wcat = np.concatenate([np.asarray(Wmu, np.float32),
                           np.asarray(Wls, np.float32)], axis=1)
    bcat = np.concatenate([np.asarray(bmu, np.float32),
                           np.asarray(bls, np.float32)])
    ident = np.eye(128, dtype=np.float32)
    colofs = [np.concatenate([[0], np.cumsum(S[r])]) for r in range(nr)]
    tofs = np.concatenate([[0], np.cumsum(TS)])

    for c in range(p):
        pc = per_core[c]
        ew_grid = np.zeros((128, max(1, gcols)), dtype=np.float32)
        gidx = np.zeros(max(128, gcols * 128), dtype=np.int64)
        for r in range(nr):
            mr = pc["r"] == r
            cq = cpos[c][r][pc["q"][mr]]
            lis, ews = pc["li"][mr], pc["ew"][mr]
            o = np.argsort(cq, kind="stable")
            cq_s, li_s, ew_s = cq[o], lis[o], ews[o]
            slot = _slot_ranks(cq_s)
            cb = cq_s // 128
            part = cq_s % 128
            gcol = gofs_r[r] + colofs[r][cb] + slot
            ew_grid[part, gcol] = ew_s
            gidx[gcol * 128 + part] = li_s

        ew_tgrid = np.zeros((128, tcols), dtype=np.float32)
        qs = pc["q"]
        o = np.argsort(qs, kind="stable")
        q_s, ew_s = qs[o], pc["ew"][o]
        slot = _slot_ranks(q_s)
        ew_tgrid[q_s % 128, tofs[q_s // 128] + slot] = ew_s

        mw = np.zeros((128, nr * nb), dtype=np.float32)
        midx = np.zeros(nr * nb * 128, dtype=np.int64)
        qq = np.arange(npad)
        for r in range(nr):
            present = Ls[c][r] > 0
            mw[qq % 128, r * nb + qq // 128] = present.astype(np.float32)
            midx[r * nb * 128 + qq] = np.where(present, cpos[c][r], 0)

        x_own = np.zeros((npad, 128), dtype=np.float32)
        x_own[:sh] = x[c * sh + pis[c][:sh]]

        in_maps.append({
            "x_own": x_own,
            "ew_grid": ew_grid,
            "ew_tgrid": ew_tgrid,
            "gidx": _wrap16(gidx),
            "midx": _wrap16(midx),
            "mw": mw,
            "w1": np.asarray(W1, np.float32),
            "wcat": wcat,
            "b1row": np.asarray(b1, np.float32).reshape(1, -1),
            "bcatrow": bcat.reshape(1, -1),
            "ident": ident,
        })

    meta = dict(ncb=[int(v) for v in ncb],
                S=[list(map(int, S[r])) for r in range(nr)],
                TS=list(map(int, TS)), gcols=max(1, gcols), tcols=tcols,
                gofs_r=list(map(int, gofs_r)))
    return in_maps, meta, pis


# ----------------------------------------------------------------------------
# device program
# ----------------------------------------------------------------------------

def _build(cfg, meta, split=True):
    import concourse.bacc as bacc
    import concourse.mybir as mybir
    from concourse.tile import TileContext

    p, h, o = cfg["p"], cfg["h"], cfg["o"]
    nb, npad, nr, rs = cfg["nb"], cfg["npad"], cfg["nr"], cfg["rs"]
    tbl = cfg["tbl"]
    ncb, S, TS = meta["ncb"], meta["S"], meta["TS"]
    gcols, tcols, gofs_r = meta["gcols"], meta["tcols"], meta["gofs_r"]
    CPC, MBB = cfg["cols_per_call"], cfg["mb_batch"]
    f32, i16 = mybir.dt.float32, mybir.dt.int16
    AX = mybir.AxisListType.X
    OP = mybir.AluOpType
    ACTF = mybir.ActivationFunctionType

    nc = bacc.Bacc(num_devices=p)
    ew_grid = nc.declare_dram_parameter("ew_grid", [128, gcols], f32, isOutput=False)
    ew_tgrid = nc.declare_dram_parameter("ew_tgrid", [128, tcols], f32, isOutput=False)
    x_own = nc.declare_dram_parameter("x_own", [npad, 128], f32, isOutput=False)
    gidx = nc.declare_dram_parameter("gidx", [128, gcols * 8], i16, isOutput=False)
    midx = nc.declare_dram_parameter("midx", [128, nr * nb * 8], i16, isOutput=False)
    mw = nc.declare_dram_parameter("mw", [128, nr * nb], f32, isOutput=False)
    w1 = nc.declare_dram_parameter("w1", [128, h], f32, isOutput=False)
    wcat = nc.declare_dram_parameter("wcat", [128, 2 * o], f32, isOutput=False)
    b1row = nc.declare_dram_parameter("b1row", [1, h], f32, isOutput=False)
    bcatrow = nc.declare_dram_parameter("bcatrow", [1, 2 * o], f32, isOutput=False)
    ident = nc.declare_dram_parameter("ident", [128, 128], f32, isOutput=False)
    out_ext = nc.declare_dram_parameter("out", [npad, 128], f32, isOutput=True)

    with TileContext(nc) as tc:
        with tc.tile_pool(name="dram", bufs=1, space="DRAM") as dram, \
             tc.tile_pool(name="persist", bufs=1) as pp, \
             tc.tile_pool(name="own", bufs=1) as ownp, \
             tc.tile_pool(name="gix", bufs=2) as gixp, \
             tc.tile_pool(name="grid", bufs=cfg["grid_bufs"]) as gp, \
             tc.tile_pool(name="mg", bufs=cfg["mg_bufs"]) as mgp, \
             tc.tile_pool(name="work", bufs=cfg["work_bufs"]) as wp, \
             tc.tile_pool(name="psum", bufs=4, space="PSUM") as psp:

            shard_t = dram.tile([npad, 128], f32, tag="shard")
            shared = "Shared" if p > 4 else "Local"
            full1_t = dram.tile([tbl, 128], f32, tag="full1", addr_space=shared)
            full2_t = dram.tile([tbl, 128], f32, tag="full2", addr_space=shared)
            partials = [dram.tile([ncb[r] * 128, 128], f32, tag=f"part{r}",
                                  name=f"part{r}") for r in range(nr)]

            ewg_t = pp.tile([128, gcols], f32, tag="ewg")
            midx_t = pp.tile([128, nr * nb * 8], i16, tag="midx")
            gix_cols = max(sum(S[r]) for r in range(nr)) * 8
            ewt_t = gixp.tile([128, tcols], f32, tag="gix", name="ewt_t")
            mw_t = pp.tile([128, nr * nb], f32, tag="mw")
            w1_t = pp.tile([128, h], f32, tag="w1")
            wcat_t = pp.tile([128, 2 * o], f32, tag="wcat")
            b1_t = pp.tile([1, h], f32, tag="b1")
            bcat_t = pp.tile([1, 2 * o], f32, tag="bcat")
            id_t = pp.tile([128, 128], f32, tag="id")
            ones_t = pp.tile([1, 128], f32, tag="ones")
            deg_t = pp.tile([128, nb], f32, tag="deg")
            d2_t = pp.tile([128, nb], f32, tag="d2")
            dv_t = pp.tile([128, nb], f32, tag="dv")
            dvrow_t = pp.tile([128, 128], f32, tag="dvrow")
            dvcat_t = pp.tile([1, nb * 128], f32, tag="dvcat")

            for t, src in [(ewg_t, ew_grid), (midx_t, midx), (mw_t, mw),
                           (w1_t, w1), (wcat_t, wcat), (b1_t, b1row),
                           (bcat_t, bcatrow), (id_t, ident)]:
                nc.sync.dma_start(out=t[:], in_=src[:])
            nc.sync.dma_start(out=ewt_t[:, :tcols], in_=ew_tgrid[:])
            nc.vector.memset(ones_t[:], 1.0)

            # deg -> 1/deg (= dinv^2) and dinv --------------------------------
            tof = 0
            for b in range(nb):
                if TS[b] > 0:
                    nc.vector.tensor_reduce(deg_t[:, b:b + 1],
                                            ewt_t[:, tof:tof + TS[b]],
                                            axis=AX, op=OP.add)
                else:
                    nc.vector.memset(deg_t[:, b:b + 1], 0.0)
                tof += TS[b]
            nc.vector.tensor_scalar_add(deg_t[:], deg_t[:], 1.0)
            nc.vector.reciprocal(d2_t[:], deg_t[:])
            nc.scalar.sqrt(dv_t[:], d2_t[:])
            # dinv rows at partition 0 (for the bias outer-product lhsT)
            dvr_ps = psp.tile([128, 128], f32, tag="ps")
            nc.tensor.transpose(dvr_ps[:nb, :], dv_t[:, :nb], id_t[:])
            nc.scalar.activation(dvrow_t[:nb, :], dvr_ps[:nb, :], ACTF.Copy)
            nc.sync.dma_start(
                out=dvcat_t[:].rearrange("p (b c) -> p b c", c=128)[0:1, :nb, :],
                in_=dvrow_t[:nb, :])

            # xs_own = dinv * x_own ; write shard -----------------------------
            own_tiles = []
            for b in range(nb):
                xt = ownp.tile([128, 128], f32, tag=f"own{b}")
                nc.sync.dma_start(out=xt[:], in_=x_own[b * 128:(b + 1) * 128, :])
                nc.vector.tensor_scalar_mul(xt[:], xt[:], dv_t[:, b:b + 1])
                nc.sync.dma_start(out=shard_t[b * 128:(b + 1) * 128, :], in_=xt[:])
                own_tiles.append(xt)

            groups = [list(range(p))]

            def allgather(dst):
                tc.strict_bb_all_engine_barrier()
                nc.gpsimd.collective_compute(
                    "AllGather", OP.bypass, replica_groups=groups,
                    ins=[shard_t.opt()], outs=[dst.opt()])
                tc.strict_bb_all_engine_barrier()

            # column -> compact-block map per range
            col2cb = []
            for r in range(nr):
                m = []
                for cb in range(ncb[r]):
                    m += [cb] * S[r][cb]
                col2cb.append(m)

            def layer(table, front_cols, wmat, bias_lhsT, bias_rhs, relu, out_writer):
                # aggregation into per-range partials
                for r in range(nr):
                    for cb in range(ncb[r]):
                        if S[r][cb] == 0:
                            zt = wp.tile([128, 128], f32, tag="pt")
                            nc.vector.memset(zt[:], 0.0)
                            nc.sync.dma_start(
                                out=partials[r][cb * 128:(cb + 1) * 128, :],
                                in_=zt[:])
                    total_cols = sum(S[r])
                    git = gixp.tile([128, gix_cols], i16, tag="gix",
                                    name=f"git_{r}")
                    nc.sync.dma_start(out=git[:, :total_cols * 8],
                                      in_=gidx[:, gofs_r[r] * 8:
                                               (gofs_r[r] + total_cols) * 8])
                    done = 0
                    pt = None
                    first = True
                    while done < total_cols:
                        ncall = min(CPC, total_cols - done)
                        grid = gp.tile([128, CPC * 128], f32, tag="grid")
                        nc.gpsimd.dma_gather(
                            out_ap=grid[:, :ncall * 128].rearrange(
                                "p (g c) -> p g c", c=128),
                            in_ap=table[r * rs:(r + 1) * rs, :],
                            idxs_ap=git[:, done * 8:(done + ncall) * 8],
                            num_idxs=ncall * 128, num_idxs_reg=ncall * 128,
                            elem_size=128, single_packet=False)
                        for j in range(ncall):
                            lcol = done + j
                            cb = col2cb[r][lcol]
                            gcol = gofs_r[r] + lcol
                            if pt is None:
                                pt = wp.tile([128, 128], f32, tag="pt")
                                first = True
                            src = grid[:, j * 128:(j + 1) * 128]
                            sc = ewg_t[:, gcol:gcol + 1]
                            if first:
                                nc.vector.tensor_scalar_mul(pt[:], src, sc)
                                first = False
                            else:
                                nc.vector.scalar_tensor_tensor(
                                    pt[:], src, sc, pt[:], OP.mult, OP.add)
                            last_of_cb = (lcol + 1 == total_cols
                                          or col2cb[r][lcol + 1] != cb)
                            if last_of_cb:
                                nc.sync.dma_start(
                                    out=partials[r][cb * 128:(cb + 1) * 128, :],
                                    in_=pt[:])
                                pt = None
                        done += ncall

                # merge + dense epilogue, batched over final blocks
                for b0 in range(0, nb, MBB):
                    nbb = min(MBB, nb - b0)
                    mgs = []
                    for r in range(nr):
                        mg = mgp.tile([128, MBB * 128], f32, tag=f"mg{r}")
                        s0 = (r * nb + b0) * 128
                        nc.gpsimd.dma_gather(
                            out_ap=mg[:, :nbb * 128].rearrange(
                                "p (g c) -> p g c", c=128),
                            in_ap=partials[r][:],
                            idxs_ap=midx_t[:, s0 // 16:(s0 + nbb * 128) // 16],
                            num_idxs=nbb * 128, num_idxs_reg=nbb * 128,
                            elem_size=128, single_packet=False)
                        mgs.append(mg)
                    for bi in range(nbb):
                        b = b0 + bi
                        agg = wp.tile([128, 128], f32, tag="agg")
                        prev = own_tiles[b]
                        for r in range(nr):
                            nc.vector.scalar_tensor_tensor(
                                agg[:], mgs[r][:, bi * 128:(bi + 1) * 128],
                                mw_t[:, r * nb + b:r * nb + b + 1],
                                prev[:], OP.mult, OP.add)
                            prev = agg
                        asc = wp.tile([128, 128], f32, tag="asc")
                        nc.scalar.activation(asc[:], agg[:], ACTF.Copy,
                                             scale=front_cols[:, b:b + 1])
                        tps = psp.tile([128, 128], f32, tag="ps")
                        nc.tensor.transpose(tps[:], asc[:], id_t[:])
                        aggT = wp.tile([128, 128], f32, tag="aggT")
                        nc.scalar.activation(aggT[:], tps[:], ACTF.Copy)
                        zps = psp.tile([128, 128], f32, tag="zps")
                        nc.tensor.matmul(zps[:], bias_lhsT(b), bias_rhs[:],
                                         start=True, stop=False)
                        nc.tensor.matmul(zps[:], aggT[:], wmat[:],
                                         start=False, stop=True)
                        res = wp.tile([128, 128], f32, tag="res")
                        nc.scalar.activation(res[:], zps[:],
                                             ACTF.Relu if relu else ACTF.Copy)
                        out_writer(b, res)

            # ---- layer 1: hs = relu(dinv^2*aggraw@W1 + dinv x b1) ----
            allgather(full1_t)

            def l1_write(b, res):
                nc.vector.tensor_copy(own_tiles[b][:], res[:])
                nc.sync.dma_start(out=shard_t[b * 128:(b + 1) * 128, :],
                                  in_=res[:])

            layer(full1_t, d2_t, w1_t,
                  lambda b: dvcat_t[:].rearrange("p (b c) -> p b c", c=128)[0:1, b, :],
                  b1_t, True, l1_write)

            # ---- layers 2+3: [mu|ls] = dinv*agg2raw@[Wmu|Wls] + [bmu|bls] ----
            allgather(full2_t)

            def l2_write(b, res):
                nc.sync.dma_start(out=out_ext[b * 128:(b + 1) * 128, :],
                                  in_=res[:])

            layer(full2_t, dv_t, wcat_t, lambda b: ones_t[:], bcat_t, False, l2_write)

    nc.finalize()
    if split:
        _split_excess_waits(nc)
    return nc


# ----------------------------------------------------------------------------
# top-level entry
# ----------------------------------------------------------------------------

_CACHE = {}


def get_built(cfg, meta):
    key = repr((sorted(cfg.items()), repr(meta)))
    if key not in _CACHE:
        _CACHE[key] = _build(cfg, meta)
    return _CACHE[key]


def run(inputs, cfg):
    from concourse.bass_utils import run_bass_kernel_spmd
    in_maps, meta, pis = _preprocess(cfg, **inputs)
    nc = get_built(cfg, meta)
    res = run_bass_kernel_spmd(nc, in_maps, list(range(cfg["p"])))
    return postprocess(res.results, pis, cfg)


def postprocess(results, pis, cfg):
    n, sh, o, p = cfg["n"], cfg["sh"], cfg["o"], cfg["p"]
    mu = np.empty((n, o), dtype=np.float32)
    ls = np.empty((n, o), dtype=np.float32)
    for c in range(p):
        out = results[c]["out"]
        pi = pis[c]
        real = pi < sh
        mu[c * sh + pi[real]] = out[real][:, :o]
        ls[c * sh + pi[real]] = out[real][:, o:2 * o]
    return mu, ls


def kernel(x, edge_index, edge_attr, W1, b1, Wmu, bmu, Wls, bls):
    cfg = _make_cfg(**CFG_PROD)
    return run(dict(x=x, edge_index=edge_index, edge_attr=edge_attr, W1=W1,
                    b1=b1, Wmu=Wmu, bmu=bmu, Wls=Wls, bls=bls), cfg)

